# revision 1
# baseline (speedup 1.0000x reference)
"""Trainium2 Bass kernel for the Critic model (attention-pointer critic).

Math (per batch b, with coords = raw-reshape(static[b]) as [2, N]):
    sh  = enc_w @ coords + enc_b                       [H, N]
    for layer i in 1..3:
        e_i  = ref_wi @ sh + ref_bi                    [H, N]
        q_i  = q_wi @ hy + q_bi                        [H]
        u_i  = v_i . tanh(e_i + q_i)                   [N]
        p_i  = softmax(u_i)
        hy   = e_i @ p_i                               [H]
    out = fc2 @ relu(fc1 @ hy + fc1_b) + fc2_b         [1]

Everything upstream/downstream of the nonlinearities is linear in coords, so
fold on-device:
    W'_i  = ref_wi @ enc_w           [H, 2]
    b'_i  = ref_wi @ enc_b + ref_bi  [H]
    e_i   = W'_i @ coords + b'_i
    hy_i  = W'_i @ z_i + b'_i   where  z_i = coords @ p_i  (a 2-vector!)
    q_{i+1} = (q_w_{i+1} @ W'_i) @ z_i + (q_w_{i+1} @ b'_i + q_b_{i+1})
    fc1 @ hy_3 + fc1_b = (fc1 @ W'_3) @ z_3 + (fc1 @ b'_3 + fc1_b)

So the only O(H*N) work per (batch, layer) is:
    e = W' @ coords        K=2 matmul on PE (fp32r, full rate at N>=256)
    t = tanh(e + qeff)     ACT, per-partition bias  (the bottleneck engine)
    u = v . t              K=H matmul on PE via one-hot-masked v weights,
                           accumulating u into row b of a [32, N] psum tile
and per layer (batched over the 32 local batches, b in partitions):
    softmax over N on DVE/ACT, z = (p*X).sum / sum(p) via fused mul-reduce.

Hardware constraints honored here:
  * PE operand/output base partitions must be 0/32/64 -> coords live as
    [2, group, N] tiles (batch in free dim); u rows land via masked weights.
  * This walrus build allows AT MOST ONE sync wait per instruction struct:
    _split_multi_waits post-processes the scheduled BIR, hoisting extra
    waits onto standalone InstEventSemaphore instructions (engines are
    in-order, so semantics are identical), chunking semaphore range-clears
    to <= 8 sems, and stripping embedded sync from custom DVE ops.
  * Scheduling: every engine's program order is fixed at emission priority,
    so all weight DMAs/folding are emitted lazily next to their consumers
    (layer-1 path first; later layers' prep is emitted mid-loop to fill PE
    slack under the ACT-bound steady state).

Sharding: pure data-parallel, 32 batches per core across 8 cores; all
weights replicated. ACT (tanh) is the roofline at ~200us busy/core.
"""

import sys

if "/opt/trn_rl_repo" not in sys.path:
    sys.path.insert(0, "/opt/trn_rl_repo")

from contextlib import ExitStack

import numpy as np

import concourse.bass as bass
import concourse.tile as tile
from concourse import mybir
from concourse.bass import _add_dep_helper
from concourse.bass_utils import run_bass_kernel_spmd


def _order(after, before):
    """Force `after` to schedule after `before` (same-engine order, no sem)."""
    _add_dep_helper(after.ins, before.ins, sync=False, reason="wait-budget order")

B, N, H = 256, 1000, 256
NCORES = 8
BC = B // NCORES  # batches per core
GB = 8            # batches per coords tile

F32 = mybir.dt.float32
F32R = mybir.dt.float32r
AF = mybir.ActivationFunctionType
ALU = mybir.AluOpType
AX = mybir.AxisListType

# PSUM bank = 2KB = 512 fp32; matmul output must stay within one bank.
NCH = (0, 512, 1000)


def _split_multi_waits(nc):
    """Walrus in this container accepts at most one sync wait per
    instruction struct. Hoist extra waits onto standalone InstEventSemaphore
    instructions inserted just before the owner (engines are in-order, so the
    semantics are identical)."""
    import os
    split_max = int(os.environ.get("SPLIT_MAX", "999999"))
    nsofar = [0]

    def mk_ev(inst, w):
        ev = mybir.InstEventSemaphore(name=nc.get_next_instruction_name())
        ev.engine = inst.engine
        ev.sync_info = mybir.SyncInfo(on_wait=[w], on_update=[])
        ev.debug = mybir.OpDebugInfo(
            op_name=f"splitwait:{inst.name}:{w.ant_name}",
            filename="kernel.py", lineno=1)
        nc.register_instruction(ev)
        return ev

    f = nc.m.functions[0]
    blocks = list(f.blocks)

    # EVENT_SEMAPHORE_RANGE_CLEAR supports at most 8 semaphores per
    # instruction on this walrus; chunk wider ranges.
    for blk in blocks:
        old_insts = blk.instructions
        rewritten = []
        changed = False
        for inst in old_insts:
            if (type(inst).__name__ == "InstISA"
                    and inst.op_name == "EVENT_SEMAPHORE_RANGE_CLEAR"):
                d = dict(inst.ant_dict)
                first, last = d["range_first"], d["range_last"]
                if last - first + 1 > 8:
                    changed = True
                    lo = first
                    while lo <= last:
                        hi = min(lo + 7, last)
                        nb = list(inst.instr)
                        nb[13], nb[14] = lo, hi
                        d2 = dict(d)
                        d2["range_first"], d2["range_last"] = lo, hi
                        ni = mybir.InstISA(
                            name=nc.get_next_instruction_name(),
                            isa_opcode=inst.isa_opcode,
                            engine=inst.engine,
                            instr=nb,
                            op_name=inst.op_name,
                            ins=[], outs=[],
                            ant_dict=d2,
                            verify=inst.verify,
                            ant_isa_is_sequencer_only=inst.ant_isa_is_sequencer_only,
                        )
                        if inst.sync_info is not None and lo == first:
                            ni.sync_info = inst.sync_info
                        nc.register_instruction(ni)
                        rewritten.append(ni)
                        lo = hi + 1
                    continue
            rewritten.append(inst)
        if changed:
            blk.instructions = rewritten

    for bi, blk in enumerate(blocks):
        old = blk.instructions
        if not any(i.sync_info is not None and len(i.sync_info.on_wait) > 1
                   for i in old):
            continue
        new = []
        hoist_prev = []  # evsems that must run before this block is entered
        for idx, inst in enumerate(old):
            si = inst.sync_info
            is_custom = type(inst).__name__ in ("InstReciprocal",)
            if si is not None and is_custom and (si.on_wait or si.on_update):
                # custom-DVE ops lower to fixed-length ISA payloads that
                # cannot carry embedded sync: hoist waits before, updates
                # after (engine is in-order, semantics unchanged).
                for w in si.on_wait:
                    new.append(mk_ev(inst, w))
                posts = list(si.on_update)
                inst.sync_info = mybir.SyncInfo(on_wait=[], on_update=[])
                new.append(inst)
                for u in posts:
                    ev = mybir.InstEventSemaphore(
                        name=nc.get_next_instruction_name())
                    ev.engine = inst.engine
                    ev.sync_info = mybir.SyncInfo(on_wait=[], on_update=[u])
                    ev.debug = mybir.OpDebugInfo(
                        op_name=f"splitupd:{inst.name}",
                        filename="kernel.py", lineno=1)
                    nc.register_instruction(ev)
                    new.append(ev)
                continue
            if si is not None and len(si.on_wait) > 1 and nsofar[0] < split_max:
                nsofar[0] += 1
                waits = list(si.on_wait)
                evs = [mk_ev(inst, w) for w in waits[:-1]]
                if idx == 0 and bi > 0 and type(inst).__name__ == "InstDrain":
                    # barrier-teardown block: walrus rejects extra
                    # instructions before the first drain, so run the waits
                    # at the tail of the previous block instead.
                    hoist_prev.extend(evs)
                else:
                    new.extend(evs)
                inst.sync_info = mybir.SyncInfo(on_wait=[waits[-1]],
                                                on_update=list(si.on_update))
            new.append(inst)
        blk.instructions = new
        if hoist_prev:
            prev = blocks[bi - 1]
            pinsts = prev.instructions
            cut = len(pinsts)
            while cut > 0 and "Branch" in type(pinsts[cut - 1]).__name__:
                cut -= 1
            prev.instructions = pinsts[:cut] + hoist_prev + pinsts[cut:]


def build_nc():
    nc = bass.Bass(trn_type="TRN2", target_bir_lowering=False)

    def din(name, shape):
        return nc.dram_tensor(name, shape, F32, kind="ExternalInput").ap()

    x = din("x", [2 * BC, N])      # per batch: row 2b = coords[0], 2b+1 = coords[1]
    x0 = din("x0", [BC, N])        # coords row 0, b-partition layout
    x1 = din("x1", [BC, N])        # coords row 1
    ident = din("ident", [128, 128])
    i32f = din("i32f", [1, 32 * 32])  # eye(32) flattened
    enc_w = din("enc_w", [H, 2])
    enc_b = din("enc_b", [H])
    mats = {}
    vecs = {}
    for i in (1, 2, 3):
        mats[f"ref_w{i}"] = din(f"ref_w{i}", [H, H])
        vecs[f"ref_b{i}"] = din(f"ref_b{i}", [H])
        if i > 1:
            mats[f"q_w{i}"] = din(f"q_w{i}", [H, H])
        vecs[f"q_b{i}"] = din(f"q_b{i}", [H])
        vecs[f"v{i}"] = din(f"v{i}", [H])
    mats["fc1_w"] = din("fc1_w", [H, H])
    vecs["fc1_b"] = din("fc1_b", [H])
    fc2_w = din("fc2_w", [1, H])
    fc2_b = din("fc2_b", [1])
    out = nc.dram_tensor("out", [BC], F32, kind="ExternalOutput").ap()

    with ExitStack() as ctx:
        tc = ctx.enter_context(tile.TileContext(nc))
        const = ctx.enter_context(tc.tile_pool(name="const", bufs=1))
        tp = ctx.enter_context(tc.tile_pool(name="tp", bufs=8))
        work = ctx.enter_context(tc.tile_pool(name="work", bufs=2))
        scr = ctx.enter_context(tc.tile_pool(name="scr", bufs=2))
        ppool = ctx.enter_context(tc.tile_pool(name="ppool", bufs=3))
        ep = ctx.enter_context(tc.tile_pool(name="ep", bufs=2, space="PSUM"))
        up = ctx.enter_context(tc.tile_pool(name="up", bufs=1, space="PSUM"))
        sp = ctx.enter_context(tc.tile_pool(name="sp", bufs=2, space="PSUM"))

        mm = nc.tensor.matmul
        act = nc.scalar.activation
        dve = nc.vector
        dma = nc.sync.dma_start  # HWDGE: 8 parallel queues (waits get split)

        # ------------- loads: only the layer-1 critical path up front -------
        ids = const.tile([128, 128], F32, tag="ids", name="ids")
        dma(out=ids, in_=ident)
        encs = const.tile([128, 2, 2], F32, tag="encs", name="encs")
        dma(out=encs, in_=enc_w.rearrange("(j p) c -> p j c", p=128))

        vs = {}

        def vsget(key):
            if key not in vs:
                t = const.tile([128, 2], F32, tag=f"v_{key}", name=f"v_{key}")
                dma(out=t, in_=vecs[key].rearrange("(j p) -> p j", p=128))
                vs[key] = t
            return vs[key]

        encb = const.tile([128, 2], F32, tag="encb", name="encb")
        dma(out=encb, in_=enc_b.rearrange("(j p) -> p j", p=128))
        i32s = const.tile([1, 32 * 32], F32, tag="i32s", name="i32s")
        dma(out=i32s, in_=i32f)
        nat = {}

        # ------------- weight prep (emitted lazily, per consuming layer) -------
        # tr[k][p, j, c] = W[c, j*128+p]
        tr = {}

        def prep_tr(k):
            m = const.tile([128, 2, 256], F32, tag=f"nat_{k}", name=f"nat_{k}")
            mr = mats[k].rearrange("(j p) h -> p j h", p=128)
            for j in range(2):  # split: transposes of half j start after half j lands
                dma(out=m[:, j, :], in_=mr[:, j, :])
            nat[k] = m
            t = const.tile([128, 2, 256], F32, tag=f"tr_{k}", name=f"tr_{k}")
            for ji in range(2):  # ji-major: row-tile kk=0 of tr ready first
                for jo in range(2):
                    ps = sp.tile([128, 128], F32, tag="sp", name="sp")
                    nc.tensor.transpose(ps, nat[k][:, ji, jo * 128:(jo + 1) * 128], ids)
                    dve.tensor_copy(t[:, jo, ji * 128:(ji + 1) * 128], ps)
            tr[k] = t

        # WT[i] [2, 256] (f32r) = (ref_wi @ enc_w)^T   (e-matmul lhsT)
        # Wn[i] [p, kt, c]      = ref_wi @ enc_w       (natural)
        # bp[i] [128, 2]        = ref_wi @ enc_b + ref_bi
        WT, Wn, bp, mv, QT, qb = {}, {}, {}, {}, {}, {}

        def prep_ref(i):
            prep_tr(f"ref_w{i}")
            trw = tr[f"ref_w{i}"]
            ps = sp.tile([2, 256], F32, tag="sp", name="sp")
            for kk in range(2):
                mm(ps, lhsT=encs[:, kk, :], rhs=trw[:, kk, :],
                   start=(kk == 0), stop=(kk == 1))
            WT[i] = const.tile([2, 256], F32R, tag=f"WT{i}", name=f"WT{i}")
            dve.tensor_copy(WT[i], ps)

            Wn[i] = const.tile([128, 2, 2], F32, tag=f"Wn{i}", name=f"Wn{i}")
            for j in range(2):
                ps = sp.tile([128, 2], F32, tag="sp", name="sp")
                for kk in range(2):
                    mm(ps, lhsT=trw[:, kk, j * 128:(j + 1) * 128],
                       rhs=encs[:, kk, :], start=(kk == 0), stop=(kk == 1))
                dve.tensor_copy(Wn[i][:, j, :], ps)

            bp[i] = const.tile([128, 2], F32, tag=f"bp{i}", name=f"bp{i}")
            for j in range(2):
                ps = sp.tile([128, 1], F32, tag="sp", name="sp")
                for kk in range(2):
                    mm(ps, lhsT=trw[:, kk, j * 128:(j + 1) * 128],
                       rhs=encb[:, kk:kk + 1], start=(kk == 0), stop=(kk == 1))
                dve.tensor_add(bp[i][:, j:j + 1], ps, vsget(f"ref_b{i}")[:, j:j + 1])

        def prep_mv(i):
            # masked-v weights: mv[i][k, b, m] = v_i[c*128+k] * I[b==m]
            t = const.tile([128, 2, 32, 32], F32R, tag=f"mv{i}", name=f"mv{i}")
            tv = t.rearrange("p c b m -> p (c b m)")
            for c in range(2):
                zt = sp.tile([1, 128], F32, tag="sp", name="sp")
                nc.tensor.transpose(zt, vsget(f"v{i}")[:, c:c + 1], ids)
                vrow = work.tile([1, 128], F32, tag="vrow", name="vrow")
                dve.tensor_copy(vrow, zt)
                for h in range(2):
                    mp = sp.tile([128, 512], F32, tag="sp", name="sp")
                    mm(mp, lhsT=vrow, rhs=i32s[:, h * 512:(h + 1) * 512],
                       start=True, stop=True)
                    dve.tensor_copy(tv[:, (2 * c + h) * 512:(2 * c + h + 1) * 512], mp)
            mv[i] = t

        def prep_q(i):
            # QT[i] [2,256] = (q_wi @ W'_{i-1})^T; qb[i] = q_wi@b'_{i-1}+q_bi
            prep_tr(f"q_w{i}")
            trq = tr[f"q_w{i}"]
            ps = sp.tile([2, 256], F32, tag="sp", name="sp")
            for kk in range(2):
                mm(ps, lhsT=Wn[i - 1][:, kk, :], rhs=trq[:, kk, :],
                   start=(kk == 0), stop=(kk == 1))
            QT[i] = const.tile([2, 256], F32, tag=f"QT{i}", name=f"QT{i}")
            dve.tensor_copy(QT[i], ps)

            qb[i] = const.tile([128, 2], F32, tag=f"qb{i}", name=f"qb{i}")
            for j in range(2):
                ps = sp.tile([128, 1], F32, tag="sp", name="sp")
                for kk in range(2):
                    mm(ps, lhsT=trq[:, kk, j * 128:(j + 1) * 128],
                       rhs=bp[i - 1][:, kk:kk + 1], start=(kk == 0), stop=(kk == 1))
                dve.tensor_add(qb[i][:, j:j + 1], ps, vsget(f"q_b{i}")[:, j:j + 1])

        x0s = x1s = None  # created during layer-1 group 0

        # minimal prep before layer-1 compute can start
        prep_ref(1)
        qeff1 = const.tile([128, 2], F32, tag="qeff1", name="qeff1")
        dve.tensor_add(qeff1, bp[1], vsget("q_b1"))
        prep_mv(1)

        # ---------------- main layers ----------------
        Zs = None
        for li in (1, 2, 3):
            if li == 1:
                qeff_tile, qoff = qeff1, lambda c, b: c
            else:
                qeff = work.tile([128, 2 * BC], F32, tag="qeff", name="qeff")
                for c in range(2):
                    qp = sp.tile([128, BC], F32, tag="sp", name="sp")
                    mm(qp, lhsT=QT[li][:, c * 128:(c + 1) * 128], rhs=Zs,
                       start=True, stop=True)
                    dve.tensor_scalar_add(qeff[:, c * BC:(c + 1) * BC], qp,
                                          qb[li][:, c:c + 1])
                qeff_tile, qoff = qeff, lambda c, b: c * BC + b
            # ACT touch: fold the DVE tick for qeff into ACT's clock so the
            # first tanh needs only its PE wait.

            U = up.tile([BC, N], F32, tag="U", name="U")
            mvl = mv[li]
            pend_u = []
            ustate = {"first": True}

            def emit_u(t, c, b, _mvl=mvl, _U=U, _ustate=None):
                st = ustate if _ustate is None else _ustate
                first = st["first"]
                st["first"] = False
                last = (b == BC - 1 and c == 1)
                for n0, n1 in zip(NCH[:-1], NCH[1:]):
                    mm(_U[:, n0:n1], lhsT=_mvl[:, c, b, :],
                       rhs=t[:, n0:n1], start=first, stop=last)
            import os as _os
            _ng = int(_os.environ.get("NGROUPS", str(BC // GB)))
            for g in range(_ng):
                cg = scr.tile([2, GB, N], F32R, tag="cg", name="cg")
                dma(out=cg,
                    in_=x[2 * GB * g:2 * GB * (g + 1), :].rearrange(
                        "(g c) n -> c g n", c=2).bitcast(F32R))
                for gi in range(GB):
                    b = g * GB + gi
                    for c in range(2):
                        pe = ep.tile([128, N], F32, tag="e", name="e")
                        lw = WT[li][:, c * 128:(c + 1) * 128]
                        for n0, n1 in zip(NCH[:-1], NCH[1:]):
                            mm(pe[:, n0:n1], lhsT=lw, rhs=cg[:, gi, n0:n1],
                               start=True, stop=True)
                        t = tp.tile([128, N], F32R, tag="t", name="t")
                        act(t, pe, AF.Tanh,
                            bias=qeff_tile[:, qoff(c, b):qoff(c, b) + 1])
                        # defer this (b, c)'s u-matmuls two tanh ops behind:
                        # gives the tanh -> u-matmul semaphore edge slack so
                        # PE never polls a not-yet-posted semaphore
                        pend_u.append((t, c, b))
                        if len(pend_u) > 2:
                            emit_u(*pend_u.pop(0))
                if g == 0 and li == 1:
                    # softmax inputs, needed from ~the end of layer 1 on
                    x0s = const.tile([BC, N], F32, tag="x0s", name="x0s")
                    dma(out=x0s, in_=x0)
                    x1s = const.tile([BC, N], F32, tag="x1s", name="x1s")
                    dma(out=x1s, in_=x1)
                if g == 0 and li < 3:
                    # emit the next layer's weight prep here so it lands
                    # mid-layer in each engine's static order, filling PE
                    # slack under the ACT-bound steady state
                    prep_ref(li + 1)
                    prep_mv(li + 1)
                    prep_q(li + 1)

            while pend_u:
                emit_u(*pend_u.pop(0))

            # batched softmax over N + z = (p*X).sum / sum(p).
            # u = v . tanh(...) is bounded (|u| < ~4), so exp needs no
            # max-subtraction; skipping it shortens the layer boundary.
            P = ppool.tile([BC, N], F32, tag="P", name="P")
            ssum = work.tile([BC, 1], F32, tag="ssum", name="ssum")
            act(P, U, AF.Exp, accum_out=ssum)
            rinv = work.tile([BC, 1], F32, tag="rinv", name="rinv")
            dve.reciprocal(rinv, ssum)
            s0 = work.tile([BC, 1], F32, tag="s0", name="s0")
            s1 = work.tile([BC, 1], F32, tag="s1", name="s1")
            pxs = scr.tile([BC, N], F32, tag="pxs", name="pxs")
            dve.scalar_tensor_tensor(out=pxs, in0=P, scalar=1.0, in1=x0s,
                                     op0=ALU.mult, op1=ALU.mult, accum_out=s0)
            pxs2 = scr.tile([BC, N], F32, tag="pxs2", name="pxs2")
            dve.scalar_tensor_tensor(out=pxs2, in0=P, scalar=1.0, in1=x1s,
                                     op0=ALU.mult, op1=ALU.mult, accum_out=s1)
            spair = work.tile([BC, 2], F32, tag="spair", name="spair")
            dve.tensor_mul(spair[:, 0:1], s0, rinv)
            dve.tensor_mul(spair[:, 1:2], s1, rinv)
            zp = sp.tile([2, BC], F32, tag="sp", name="sp")
            nc.tensor.transpose(zp, spair, ids[0:BC, 0:BC])
            Zs = work.tile([2, BC], F32, tag="Z", name="Z")
            dve.tensor_copy(Zs, zp)

        # head weights: FT [2,256] = (fc1_w @ W'_3)^T, fb = fc1_w @ b'_3 + fc1_b
        fc2s = const.tile([128, 2], F32, tag="fc2s", name="fc2s")
        dma(out=fc2s, in_=fc2_w.rearrange("a (j p) -> p (a j)", p=128))
        fc2bs = const.tile([1, 1], F32, tag="fc2bs", name="fc2bs")
        dma(out=fc2bs, in_=fc2_b.unsqueeze(1))
        prep_tr("fc1_w")
        trf = tr["fc1_w"]
        ps = sp.tile([2, 256], F32, tag="sp", name="sp")
        for kk in range(2):
            mm(ps, lhsT=Wn[3][:, kk, :], rhs=trf[:, kk, :],
               start=(kk == 0), stop=(kk == 1))
        FT = const.tile([2, 256], F32, tag="FT", name="FT")
        dve.tensor_copy(FT, ps)
        fb = const.tile([128, 2], F32, tag="fb", name="fb")
        for j in range(2):
            ps = sp.tile([128, 1], F32, tag="sp", name="sp")
            for kk in range(2):
                mm(ps, lhsT=trf[:, kk, j * 128:(j + 1) * 128],
                   rhs=bp[3][:, kk:kk + 1], start=(kk == 0), stop=(kk == 1))
            dve.tensor_add(fb[:, j:j + 1], ps, vsget("fc1_b")[:, j:j + 1])

        # ---------------- head ----------------
        Rt = []
        for c in range(2):
            ap_ = sp.tile([128, BC], F32, tag="sp", name="sp")
            mm(ap_, lhsT=FT[:, c * 128:(c + 1) * 128], rhs=Zs, start=True, stop=True)
            r = work.tile([128, BC], F32, tag=f"R{c}", name=f"R{c}")
            act(r, ap_, AF.Relu, bias=fb[:, c:c + 1])
            Rt.append(r)
        op = sp.tile([1, BC], F32, tag="sp", name="sp")
        for c in range(2):
            mm(op, lhsT=fc2s[:, c:c + 1], rhs=Rt[c], start=(c == 0), stop=(c == 1))
        osb = work.tile([1, BC], F32, tag="osb", name="osb")
        dve.tensor_scalar_add(osb, op, fc2bs[0:1, 0:1])
        dma(out=out.unsqueeze(0), in_=osb)

    _split_multi_waits(nc)
    return nc


_NC = None


def _get_nc():
    global _NC
    if _NC is None:
        _NC = build_nc()
    return _NC


def make_in_maps(inputs):
    """Shard the full inputs into per-core in_maps for run_bass_kernel_spmd."""
    ins = {k: np.ascontiguousarray(np.asarray(v, dtype=np.float32))
           for k, v in inputs.items()}
    static = ins["static"]
    assert static.shape == (B, N, 2)
    ident = np.eye(128, dtype=np.float32)
    i32 = np.eye(32, dtype=np.float32).reshape(1, 1024)
    shared = {k: ins[k] for k in ins if k != "static" and k != "q_w1"}
    in_maps = []
    for c in range(NCORES):
        sh = np.ascontiguousarray(static[c * BC:(c + 1) * BC])
        m = dict(shared)
        xr = sh.reshape(2 * BC, N)  # raw reshape, matches the reference
        m["x"] = xr
        m["x0"] = np.ascontiguousarray(xr[0::2])
        m["x1"] = np.ascontiguousarray(xr[1::2])
        m["ident"] = ident
        m["i32f"] = i32
        in_maps.append(m)
    return in_maps


def kernel(**inputs) -> np.ndarray:
    nc = _get_nc()
    in_maps = make_in_maps(inputs)
    res = run_bass_kernel_spmd(nc, in_maps, list(range(NCORES)))
    outs = [np.asarray(res.results[c]["out"], dtype=np.float32)
            for c in range(NCORES)]
    return np.concatenate(outs, axis=0).reshape(B, 1)

